# revision 42
# baseline (speedup 1.0000x reference)
"""GATv2 classifier kernel for Trainium2, 8-core SPMD.

Strategy (v2, run-table gather):
  - Nodes dealt round-robin by descending in-degree across 8 cores; edges
    partitioned by destination so segment-softmax/aggregation stay local.
  - Per core, each destination's incoming-edge sources (self-loop first)
    form a CONTIGUOUS RUN in a DRAM table of xl rows, padded per 128-node
    bucket to K_b = nd*k rows/node (k in {4,8,16} adaptive per bucket).
    The table is built on device (xl = x_S @ Wl + bl) from a host-shipped
    per-edge-ordered copy of x; gathers then need only nd descriptors of
    k rows (k*256B) per node instead of one per edge: ~19K descriptors
    per core vs ~134K -- GpSimd desc-gen (7.9ns/desc) was the wall.
  - The table is split into one DRAM tensor per bucket-group, so int16
    gather indices stay in-window and the tile framework pipelines
    group g's gathers against group g+1's table build.
  - Per bucket [128 dst x K slots]: z = g + xr[dst] (DVE broadcast add),
    lr = Prelu(z) (ACT), e = sum_f att*lr (DVE mult+reduce), exact
    segment max via exp(e - emax) (ACT bias), p-weighted aggregation by
    in-place f16 halving tree (contiguous DVE adds).
  - logit = (sum_f agg*wo)/den - xr.wo + (bias.wo + bo); out = sigmoid.
"""

import math
import os
import sys

import numpy as np

if os.path.isdir("/opt/trn_rl_repo") and "/opt/trn_rl_repo" not in sys.path:
    sys.path.insert(0, "/opt/trn_rl_repo")

P = 128
NEG_SLOPE = 0.2
CHUNK = 512           # table rows per phase-1 chunk
ALPHA = 8.0           # gpsimd ns per gather descriptor
BETA = 5.0            # marginal ns per table slot (DVE/ACT/phase-1)
GROUP_BUCKETS = 4     # max buckets per table-group tensor


def _wrap_idx(grid):
    """[nd,128] desc grid -> dma_gather wrapped idx layout [128, nd*128/16]."""
    flat = grid.reshape(-1).astype(np.int16)
    return np.tile(flat.reshape(-1, 16).T, (8, 1))


# --------------------------------------------------------------------------
# Host-side planning
# --------------------------------------------------------------------------

def _plan(x, edge_index, Wl, bl, Wr, br, att, bias, Wo, bo, n_cores=8):
    N, F = x.shape
    assert F == P
    C = n_cores

    src = np.concatenate([np.asarray(edge_index[0], dtype=np.int64),
                          np.arange(N, dtype=np.int64)])
    dst = np.concatenate([np.asarray(edge_index[1], dtype=np.int64),
                          np.arange(N, dtype=np.int64)])
    deg = np.bincount(dst, minlength=N)          # includes self-loop
    # order edges by dst, self-loop first within each segment
    notself = np.ones(len(src), dtype=np.int8)
    notself[-N:] = 0
    e_order = np.lexsort((notself, dst))
    src_sorted = src[e_order].astype(np.int32)
    starts = np.concatenate([[0], np.cumsum(deg)]).astype(np.int64)

    # deal nodes round-robin by descending total degree
    order = np.argsort(-deg, kind="stable")
    npc = (N + C - 1) // C
    NB = (npc + P - 1) // P
    npc_pad = NB * P
    order_pad = np.full(C * npc_pad, -1, dtype=np.int64)
    order_pad[:N] = order
    core_nodes = np.stack([order_pad[c::C] for c in range(C)])  # [C, npc_pad]

    # per-core: sort by degree desc -> bucket grid
    nds_all = np.zeros((C, npc_pad), dtype=np.int64)
    degs_all = np.zeros((C, npc_pad), dtype=np.int64)
    for c in range(C):
        nodes = core_nodes[c]
        key = np.where(nodes >= 0, deg[np.maximum(nodes, 0)], -1)
        o = np.argsort(-key, kind="stable")
        nds_all[c] = nodes[o]
        degs_all[c] = np.where(nds_all[c] >= 0,
                               deg[np.maximum(nds_all[c], 0)], 0)

    # global (cross-core) per-bucket schedule: k and nd from max degree
    ks, nd_s = [], []
    for b in range(NB):
        dmax = max(1, int(degs_all[:, b * P:(b + 1) * P].max()))
        best = None
        for k in (4, 8, 16):
            nd = (dmax + k - 1) // k
            cost = nd * (ALPHA + k * BETA)
            if best is None or cost < best[0]:
                best = (cost, k, nd)
        ks.append(best[1])
        nd_s.append(best[2])

    # groups of consecutive buckets with equal k; uniform nd within a group
    # so each group's bucket compute runs as single wide ops
    groups = []       # list of (k, [bucket ids])
    for b in range(NB):
        if groups and groups[-1][0] == ks[b] and len(groups[-1][1]) < GROUP_BUCKETS:
            groups[-1][1].append(b)
        else:
            groups.append((ks[b], [b]))
    for k, bl_ in groups:
        ndg = max(nd_s[b] for b in bl_)
        for b in bl_:
            nd_s[b] = ndg
    Ks = [ks[b] * nd_s[b] for b in range(NB)]

    # table layout per group
    g_rows = []        # padded rows per group
    g_units = []
    b_rowoff = [0] * NB   # bucket row offset within its group
    b_group = [0] * NB
    for gi, (k, bl_) in enumerate(groups):
        r = 0
        for b in bl_:
            b_group[b] = gi
            b_rowoff[b] = r
            r += P * Ks[b]
        rpad = ((r + CHUNK - 1) // CHUNK) * CHUNK
        g_rows.append(rpad)
        g_units.append(rpad // k)
    Stot = sum(g_rows)
    g_coloff = np.concatenate([[0], np.cumsum(g_rows)]).astype(np.int64)

    # idx / mask / run-source layout (idx shared across cores; masks+runs per core)
    ko = [0] * NB
    io = [0] * NB
    kacc = iacc = 0
    for b in range(NB):
        ko[b] = kacc
        io[b] = iacc
        kacc += Ks[b]
        iacc += (nd_s[b] * P) // 16
    Ktot, I16 = kacc, iacc

    idx_arr = np.zeros((P, I16), dtype=np.int16)
    for b in range(NB):
        k, nd = ks[b], nd_s[b]
        ub = b_rowoff[b] // k
        grid = (ub + np.arange(P)[None, :] * nd + np.arange(nd)[:, None])
        idx_arr[:, io[b]:io[b] + (nd * P) // 16] = _wrap_idx(grid)

    mask = np.zeros((C, P, Ktot), dtype=np.float16)
    src_run = np.full((C, Stot), -1, dtype=np.int64)
    for c in range(C):
        nds = nds_all[c]
        degs = degs_all[c]
        for b in range(NB):
            K = Ks[b]
            base = g_coloff[b_group[b]] + b_rowoff[b]
            blk = nds[b * P:(b + 1) * P]
            db = degs[b * P:(b + 1) * P]
            kk = np.arange(K)
            m = (kk[None, :] < db[:, None]).astype(np.float16)
            m[blk < 0, 0] = 1.0          # dummy slot keeps den > 0
            mask[c, :, ko[b]:ko[b] + K] = m
            # runs
            for p in range(P):
                n = blk[p]
                if n < 0:
                    continue
                d = int(db[p])
                s0 = starts[n]
                src_run[c, base + p * K: base + p * K + d] = \
                    src_sorted[s0:s0 + d]

    x16 = np.asarray(x, dtype=np.float16)
    xT_S = np.zeros((C, P, Stot), dtype=np.float16)
    xdT = np.zeros((C, P, npc_pad), dtype=np.float16)
    for c in range(C):
        okr = src_run[c] >= 0
        xT_S[c][:, okr] = x16[src_run[c][okr]].T
        okn = nds_all[c] >= 0
        xdT[c][:, okn] = x16[nds_all[c][okn]].T

    # fold |att| into Wl/Wr columns; permute features so att>=0 come first.
    # e = sum_f att_f*lrelu(z_f) = sum_pos prelu(z'') - sum_neg prelu(z'')
    # with z'' = |att| ⊙ z; compensate aggregation with wo'' = wo/|att|.
    att64 = np.asarray(att, dtype=np.float64)
    perm = np.argsort(att64 < 0, kind="stable")
    npf = int((att64 >= 0).sum())
    attabs = np.maximum(np.abs(att64[perm]), 1e-3)
    wl = (np.asarray(Wl, dtype=np.float64)[:, perm] * attabs[None, :]) \
        .astype(np.float16)
    wr = (np.asarray(Wr, dtype=np.float64)[:, perm] * attabs[None, :]) \
        .astype(np.float16)
    bl_row = (np.asarray(bl, dtype=np.float64)[perm] * attabs) \
        .astype(np.float16).reshape(1, P)
    br_row = (np.asarray(br, dtype=np.float64)[perm] * attabs) \
        .astype(np.float16).reshape(1, P)
    wo_eff = (np.asarray(Wo, dtype=np.float64)[perm, 0] / attabs) \
        .astype(np.float16)
    wo_rep = np.tile(wo_eff[None, :], (P, 1))
    bo_eff = float(np.asarray(bo).reshape(-1)[0] +
                   np.asarray(bias, dtype=np.float64)
                   @ np.asarray(Wo, dtype=np.float64)[:, 0])

    cfg = dict(N=N, C=C, NB=NB, npc_pad=npc_pad, Stot=Stot,
               ks=ks, nd_s=nd_s, Ks=Ks, ko=ko, io=io,
               groups=groups, g_rows=g_rows, g_units=g_units,
               g_coloff=[int(v) for v in g_coloff],
               Ktot=Ktot, I16=I16, bo_eff=bo_eff, npf=npf,
               bl_nz=bool(np.any(np.asarray(bl) != 0)),
               br_nz=bool(np.any(np.asarray(br) != 0)))

    in_maps = []
    for c in range(C):
        in_maps.append({
            "xT_S": np.ascontiguousarray(xT_S[c]),
            "xdT": np.ascontiguousarray(xdT[c]),
            "idx": idx_arr,
            "mask": np.ascontiguousarray(mask[c]),
            "wl": wl, "wr": wr, "bl_row": bl_row, "br_row": br_row,
            "wo_rep": wo_rep,
        })
    return cfg, in_maps, nds_all


# --------------------------------------------------------------------------
# Device program
# --------------------------------------------------------------------------

def _build(cfg, lrelu_act=True, debug=False):
    import concourse.bass as bass
    import concourse.bacc as bacc
    import concourse.tile as tile
    from concourse import mybir

    f16, f32, i16 = mybir.dt.float16, mybir.dt.float32, mybir.dt.int16
    AT = mybir.ActivationFunctionType
    OP = mybir.AluOpType
    AX = mybir.AxisListType

    NB = cfg["NB"]
    npc_pad = cfg["npc_pad"]
    ks, nd_s, Ks, ko, io = cfg["ks"], cfg["nd_s"], cfg["Ks"], cfg["ko"], cfg["io"]
    groups, g_rows = cfg["groups"], cfg["g_rows"]
    g_coloff = cfg["g_coloff"]

    nc = bacc.Bacc("TRN2", target_bir_lowering=False, debug=debug,
                   num_devices=cfg["C"], num_swdge_queues=2)

    xT_S_d = nc.dram_tensor("xT_S", [P, cfg["Stot"]], f16, kind="ExternalInput")
    xdT_d = nc.dram_tensor("xdT", [P, npc_pad], f16, kind="ExternalInput")
    idx_d = nc.dram_tensor("idx", [P, cfg["I16"]], i16, kind="ExternalInput")
    mask_d = nc.dram_tensor("mask", [P, cfg["Ktot"]], f16, kind="ExternalInput")
    wl_d = nc.dram_tensor("wl", [P, P], f16, kind="ExternalInput")
    wr_d = nc.dram_tensor("wr", [P, P], f16, kind="ExternalInput")
    blr_d = nc.dram_tensor("bl_row", [1, P], f16, kind="ExternalInput")
    brr_d = nc.dram_tensor("br_row", [1, P], f16, kind="ExternalInput")
    wo_d = nc.dram_tensor("wo_rep", [P, P], f16, kind="ExternalInput")
    out_d = nc.dram_tensor("out", [npc_pad, 1], f32, kind="ExternalOutput")

    tables = [nc.dram_tensor(f"table{gi}", [g_rows[gi], P], f16)
              for gi in range(len(groups))]

    def bc(ap, pattern):
        return bass.AP(tensor=ap.tensor, offset=ap.offset,
                       ap=[list(ap.ap[0])] + [list(p) for p in pattern])

    with tile.TileContext(nc) as tc:
        with tc.tile_pool(name="const", bufs=1) as cp:
            wl_sb = cp.tile([P, P], f16, tag="wl")
            wr_sb = cp.tile([P, P], f16, tag="wr")
            blr_sb = cp.tile([1, P], f16, tag="blr")
            brr_sb = cp.tile([1, P], f16, tag="brr")
            wo_sb = cp.tile([P, P], f16, tag="wo")
            idx_sb = cp.tile([P, cfg["I16"]], i16, tag="idx")
            mask_sb = cp.tile([P, cfg["Ktot"]], f16, tag="mask")
            ones1 = cp.tile([1, P], f16, tag="ones1")
            bo_sb = cp.tile([P, 1], f32, tag="bo")
            out_sb = cp.tile([P, NB], f32, tag="outsb")
            lg_all = cp.tile([P, NB], f32, tag="lgall")
            xr_pd = cp.tile([P, NB, P], f16, tag="xrpd")

            for t, d in ((wl_sb, wl_d), (wr_sb, wr_d), (blr_sb, blr_d),
                         (brr_sb, brr_d), (wo_sb, wo_d)):
                nc.sync.dma_start(out=t, in_=d.ap())
            nc.vector.memset(ones1, 1.0)
            nc.vector.memset(bo_sb, cfg["bo_eff"])

            # ---------------- phases A+B interleaved ----------------
            with tc.tile_pool(name="pa", bufs=3) as ap_, \
                 tc.tile_pool(name="pap", bufs=4, space="PSUM") as app, \
                 tc.tile_pool(name="pas", bufs=3) as asp, \
                 tc.tile_pool(name="pbx", bufs=3) as xp, \
                 tc.tile_pool(name="pbp", bufs=4, space="PSUM") as pp, \
                 tc.tile_pool(name="pbc", bufs=3) as cvp, \
                 tc.tile_pool(name="gat", bufs=3) as gp, \
                 tc.tile_pool(name="lrp", bufs=3) as lp, \
                 tc.tile_pool(name="sm", bufs=8) as sp:

                xd_all = cp.tile([P, NB, P], f16, tag="xdall")
                wo4 = cp.tile([P, 4, P], f16, tag="wo4")
                nc.vector.tensor_copy(wo4, bc(wo_sb, [[0, 4], [1, P]]))

                def emit_late_consts():
                    nc.sync.dma_start(out=idx_sb, in_=idx_d.ap())
                    nc.sync.dma_start(out=mask_sb, in_=mask_d.ap())
                    nsl = (NB + 7) // 8
                    for j0 in range(0, NB, nsl):
                        j1 = min(NB, j0 + nsl)
                        nc.sync.dma_start(out=xd_all[:, j0:j1, :],
                                          in_=xdT_d.ap()[:, j0 * P:j1 * P])

                def emit_phase_a(bl):
                    for i0 in range(0, len(bl), 4):
                        bb = bl[i0:i0 + 4]
                        ps = app.tile([P, len(bb), P], f32, tag="psA")
                        for i, b in enumerate(bb):
                            if cfg["br_nz"]:
                                nc.tensor.matmul(ps[:, i, :], xd_all[:, b, :],
                                                 wr_sb, start=True, stop=False)
                                nc.tensor.matmul(ps[:, i, :], ones1, brr_sb,
                                                 start=False, stop=True)
                            else:
                                nc.tensor.matmul(ps[:, i, :], xd_all[:, b, :],
                                                 wr_sb, start=True, stop=True)
                        if bb == sorted(bb) and bb[-1] - bb[0] == len(bb) - 1:
                            nc.scalar.copy(xr_pd[:, bb[0]:bb[-1] + 1, :], ps)
                        else:
                            for i, b in enumerate(bb):
                                nc.scalar.copy(xr_pd[:, b, :], ps[:, i, :])

                def chunk_jobs(gi):
                    goff = g_coloff[gi]
                    n_ch = g_rows[gi] // CHUNK
                    for c0 in range(0, n_ch, 2):
                        yield gi, goff, n_ch, c0

                def emit_chunk_pair(job):
                    gi, goff, n_ch, c0 = job
                    if True:
                        nch = min(2, n_ch - c0)
                        w = nch * CHUNK
                        xt = xp.tile([P, nch * 4, P], f16, tag="xt")
                        nc.sync.dma_start(
                            out=xt,
                            in_=xT_S_d.ap()[:, goff + c0 * CHUNK:
                                            goff + c0 * CHUNK + w])
                        cv = cvp.tile([P, nch * 4, P], f16, tag="cv")
                        for j in range(nch):
                            ps = pp.tile([P, 4, P], f32, tag="psB")
                            for i in range(4):
                                if cfg["bl_nz"]:
                                    nc.tensor.matmul(
                                        ps[:, i, :], xt[:, 4 * j + i, :],
                                        wl_sb, start=True, stop=False)
                                    nc.tensor.matmul(
                                        ps[:, i, :], ones1, blr_sb,
                                        start=False, stop=True)
                                else:
                                    nc.tensor.matmul(
                                        ps[:, i, :], xt[:, 4 * j + i, :],
                                        wl_sb, start=True, stop=True)
                            nc.scalar.copy(cv[:, 4 * j:4 * j + 4, :], ps)
                        dstap = tables[gi].ap()[c0 * CHUNK:c0 * CHUNK + w, :]
                        nc.scalar.dma_start(
                            out=dstap.rearrange("(i n) f -> n i f", n=P),
                            in_=cv)

                def emit_group(gi, k, bl_):
                    b0 = bl_[0]
                    nb = len(bl_)
                    assert nb <= 2
                    nd, K = nd_s[b0], Ks[b0]
                    KT = nb * K
                    npf = cfg["npf"]
                    g_t = gp.tile([P, KT, P], f16, tag="g")
                    tab = tables[gi].ap()
                    in_ap = bass.AP(tensor=tab.tensor, offset=tab.offset,
                                    ap=[[k * P, g_rows[gi] // k],
                                        [1, k * P]])
                    ndt = nb * nd
                    for c0 in range(0, ndt, 8):
                        cc = min(8, ndt - c0)
                        out_ap = bass.AP(
                            tensor=g_t.tensor,
                            offset=g_t.offset + c0 * k * P,
                            ap=[list(g_t.ap[0]), [k * P, cc], [1, k * P]])
                        nc.gpsimd.dma_gather(
                            out_ap=out_ap, in_ap=in_ap,
                            idxs_ap=idx_sb[:, io[b0] + c0 * 8:
                                           io[b0] + (c0 + cc) * 8],
                            num_idxs=cc * P, num_idxs_reg=cc * P,
                            elem_size=k * P, queue_num=(gi + c0 // 8) % 2)

                    # zw_k = sum_f g_raw_k * wo (xr part cancels in logit)
                    lr = lp.tile([P, KT, P], f16, tag="lr")
                    KT4 = KT // 4
                    wo_b = bass.AP(tensor=wo4.tensor, offset=wo4.offset,
                                   ap=[list(wo4.ap[0]), [0, KT4], [1, 4 * P]])
                    g_v = bass.AP(tensor=g_t.tensor, offset=g_t.offset,
                                  ap=[list(g_t.ap[0]), [4 * P, KT4],
                                      [1, 4 * P]])
                    lr_v = bass.AP(tensor=lr.tensor, offset=lr.offset,
                                   ap=[list(lr.ap[0]), [4 * P, KT4],
                                       [1, 4 * P]])
                    nc.vector.tensor_mul(lr_v, g_v, wo_b)
                    zw = sp.tile([P, KT], f32, tag="zw")
                    nc.vector.reduce_sum(out=zw, in_=lr, axis=AX.X)
                    # z'' = g + xr[dst]; xr pre-expanded 4x for wide runs
                    K4 = K // 4
                    xr4 = sp.tile([P, nb, 4, P], f16, tag="xr4")
                    xs = xr_pd[:, b0, :]
                    nc.vector.tensor_copy(
                        xr4, bass.AP(tensor=xs.tensor, offset=xs.offset,
                                     ap=[list(xs.ap[0]), [P, nb],
                                         [0, 4], [1, P]]))
                    g8 = bass.AP(tensor=g_t.tensor, offset=g_t.offset,
                                 ap=[list(g_t.ap[0]), [K * P, nb],
                                     [4 * P, K4], [1, 4 * P]])
                    xr_b = bass.AP(tensor=xr4.tensor, offset=xr4.offset,
                                   ap=[list(xr4.ap[0]), [4 * P, nb],
                                       [0, K4], [1, 4 * P]])
                    nc.vector.tensor_add(g8, g8, xr_b)
                    # att>=0 features: lr = prelu(z); att<0 features use
                    # -prelu(v) = prelu_{alpha=5}(-0.2*v), so e is ONE
                    # full-width reduce.
                    if lrelu_act:
                        if npf > 0:
                            nc.scalar.activation(lr[:, :, 0:npf],
                                                 g_t[:, :, 0:npf],
                                                 AT.Prelu, alpha=NEG_SLOPE)
                        if npf < P:
                            nc.scalar.activation(lr[:, :, npf:P],
                                                 g_t[:, :, npf:P],
                                                 AT.Prelu, alpha=1.0 / NEG_SLOPE,
                                                 scale=-NEG_SLOPE)
                    else:
                        nc.vector.scalar_tensor_tensor(
                            out=lr, in0=g_t, scalar=NEG_SLOPE, in1=g_t,
                            op0=OP.mult, op1=OP.max)
                    e_t = sp.tile([P, KT], f32, tag="e")
                    if not lrelu_act and 0 < npf < P:
                        en = sp.tile([P, KT], f32, tag="en")
                        nc.vector.reduce_sum(out=e_t,
                                             in_=lr[:, :, 0:npf], axis=AX.X)
                        nc.vector.reduce_sum(out=en,
                                             in_=lr[:, :, npf:P], axis=AX.X)
                        nc.vector.tensor_sub(e_t, e_t, en)
                    else:
                        nc.vector.reduce_sum(out=e_t, in_=lr, axis=AX.X)
                    # exact masking: e_m = (e+100)*mask
                    em = sp.tile([P, KT], f32, tag="em")
                    nc.vector.scalar_tensor_tensor(
                        out=em, in0=e_t, scalar=100.0,
                        in1=mask_sb[:, ko[b0]:ko[b0] + KT],
                        op0=OP.add, op1=OP.mult)
                    nmax = sp.tile([P, nb], f32, tag="nmax")
                    em3 = bass.AP(tensor=em.tensor, offset=em.offset,
                                  ap=[list(em.ap[0]), [K, nb], [1, K]])
                    nc.vector.reduce_max(out=nmax, in_=em3, axis=AX.X,
                                         negate=True)
                    pp_t = sp.tile([P, KT], f32, tag="pp")
                    den = sp.tile([P, nb], f32, tag="den")
                    rden = sp.tile([P, nb], f32, tag="rden")
                    scr = sp.tile([P, K], f32, tag="scr")
                    for j, b in enumerate(bl_):
                        nc.scalar.activation(pp_t[:, j * K:(j + 1) * K],
                                             em[:, j * K:(j + 1) * K],
                                             AT.Exp, bias=nmax[:, j:j + 1],
                                             accum_out=den[:, j:j + 1])
                    nc.vector.reciprocal(rden, den)
                    for j, b in enumerate(bl_):
                        aw = sp.tile([P, 1], f32, tag="aw")
                        nc.vector.scalar_tensor_tensor(
                            out=scr, in0=pp_t[:, j * K:(j + 1) * K],
                            scalar=1.0, in1=zw[:, j * K:(j + 1) * K],
                            op0=OP.mult, op1=OP.mult, accum_out=aw)
                        nc.vector.tensor_scalar_mul(
                            lg_all[:, b:b + 1], aw, rden[:, j:j + 1])

                # ---- driver: smallest groups first; chunk work is
                # ---- drained AFTER each compute pair so prelus never queue
                # ---- behind future groups' copies on the scalar engine
                from collections import deque
                order = sorted(range(len(groups)),
                               key=lambda gi: g_rows[gi] // max(
                                   1, len(groups[gi][1])))
                emit_late_consts()
                early = [b for gi in order[:2] for b in groups[gi][1]]
                emit_phase_a(early)
                for gi in order[:2]:
                    for job in chunk_jobs(gi):
                        emit_chunk_pair(job)
                emit_phase_a([b for b in range(NB) if b not in early])
                chunk_q = deque()
                for gi in order[2:]:
                    chunk_q.extend(chunk_jobs(gi))
                npairs = sum((len(groups[gi][1]) + 1) // 2 for gi in order)
                per = max(1, -(-len(chunk_q) // max(1, npairs - 6)))
                for oi, gi in enumerate(order):
                    k, bl_ = groups[gi]
                    for p0 in range(0, len(bl_), 2):
                        emit_group(gi, k, bl_[p0:p0 + 2])
                        for _ in range(per):
                            if chunk_q:
                                emit_chunk_pair(chunk_q.popleft())
                while chunk_q:
                    emit_chunk_pair(chunk_q.popleft())

            nc.scalar.activation(out_sb, lg_all, AT.Sigmoid, bias=bo_sb)
            nc.sync.dma_start(
                out=out_d.ap().rearrange("(b n) o -> n (b o)", n=P),
                in_=out_sb)
    nc.compile()
    return nc


# --------------------------------------------------------------------------
# Entry point
# --------------------------------------------------------------------------

def _run(inputs, trace=False, lrelu_act=True):
    from concourse.bass_utils import run_bass_kernel_spmd

    cfg, in_maps, out_nodes = _plan(**inputs)
    nc = _build(cfg, lrelu_act=lrelu_act)
    res = run_bass_kernel_spmd(nc, in_maps, core_ids=list(range(cfg["C"])),
                               trace=trace)

    N = cfg["N"]
    out = np.zeros((N, 1), dtype=np.float32)
    for c in range(cfg["C"]):
        nodes = out_nodes[c]
        ok = nodes >= 0
        out[nodes[ok], 0] = res.results[c]["out"][ok, 0]
    return out, res


def kernel(**inputs):
    return _run(inputs)[0]
